# revision 24
# baseline (speedup 1.0000x reference)
"""Multi-head attention (no softmax) on 8 trn2 NeuronCores.

Reference: out = ((x @ Wqkv.T -> q,k,v per head) ; (q @ k.T * s) @ v ; concat ; @ Wproj.T)

Because there is no softmax the attention is linear:
    (q @ k.T) @ v == q @ (k.T @ v),  k.T @ v is only 64x64 per head,
so the T x T score matrices never need to exist. Per head:
    M_h = (s * k_h).T @ v_h        (64 x 64, reduced over ALL tokens of the batch)
    out += (q_h @ M_h) @ Wproj_h.T

Sharding: token-parallel. Core c owns batch b=c//2, token half c%2 (512 tokens).
M_h needs a reduction over the full batch -> one 128KB AllReduce(add) between
the two cores of each batch.

Everything runs in bf16 (same PE rate as fp32r, half the HBM traffic; rel err
~5e-3 vs the 2e-2 gate). PSUM accumulates fp32. The 1/8 head scale is folded
into W_k on the host (exact).

Collective physics on this platform (measured): every collective op is a
global 8-core rendezvous; the FIRST op carries ~13us of semaphore hops that
freeze while the DMA engines are saturated, and its duration absorbs the
skew between cores; subsequent ops start ~1-2us after the previous and take
~6-10us. So: a dummy 256B AllReduce is triggered at t~8 to pre-pay the
rendezvous (its hops run right after the 6MB phase-1 bulk drains at ~32us),
and the real M AllReduce (both halves at once) chains behind it warm.
W_proj (2MB) is deferred to the scalar queue so the phase-1 drain is early.

DMAs are coarse - the host pre-swizzles every operand into its exact
[128, cols] SBUF layout so each logical group is ONE contiguous dma_start
(the Sync sequencer spends ~0.6us of issue time per dma_start; the f32
baseline burned ~40us there on 64 transfers). Dummy matmuls at t=0 and
during the collective wait keep the PE out of its low p-state.
"""

import numpy as np

B, T, E = 4, 1024, 1024
NH, HD = 16, 64
N_CORES = 8
TPC = T // 2  # tokens per core = 512

_built = None


def _build():
    """Build + compile the 8-core SPMD Bass program once."""
    global _built
    if _built is not None:
        return _built

    import concourse.mybir as mybir
    import concourse.tile as tile
    from concourse import bacc

    f32 = mybir.dt.float32
    bf16 = mybir.dt.bfloat16
    GROUPS = [[0, 1], [2, 3], [4, 5], [6, 7]]

    nc = bacc.Bacc("TRN2", target_bir_lowering=False, debug=False, num_devices=N_CORES)
    # x pre-swizzled: [128 part, e*512 + tok]
    xd = nc.dram_tensor("xd", [128, 4096], bf16, kind="ExternalInput").ap()
    # kv weights: 4 stream groups (k0, v0, k1, v1), each [128 part, e*512 + kvf]
    wkvd = nc.dram_tensor("wkvd", [4 * 128, 8 * 512], bf16, kind="ExternalInput").ap()
    # q weights: 2 column-half groups, each [128 part, e*512 + qf]
    wqd = nc.dram_tensor("wqd", [2 * 128, 8 * 512], bf16, kind="ExternalInput").ap()
    # proj weights, oc-major: [128 part, oc*4096 + f*512 + c]
    wpd = nc.dram_tensor("wpd", [128, 8 * 1024], bf16, kind="ExternalInput").ap()
    out = nc.dram_tensor("out", [TPC, E], f32, kind="ExternalOutput").ap()

    evict_i = [0]

    def evict(dst, src):
        # spread PSUM->SBUF eviction copies across DVE and ACT
        if evict_i[0] % 2 == 0:
            nc.vector.tensor_copy(dst, src)
        else:
            nc.scalar.copy(dst, src)
        evict_i[0] += 1

    with tile.TileContext(nc) as tc:
        with (
            tc.tile_pool(name="xp", bufs=1) as xp,
            tc.tile_pool(name="wkvp", bufs=1) as wkvp,
            tc.tile_pool(name="kvp", bufs=1) as kvp,
            tc.tile_pool(name="wqp", bufs=1) as wqp,
            tc.tile_pool(name="wpp", bufs=1) as wpp,
            tc.tile_pool(name="qp", bufs=1) as qp,
            tc.tile_pool(name="mres", bufs=1) as mres,
            tc.tile_pool(name="op", bufs=2) as op,
            tc.tile_pool(name="warm", bufs=1) as warmp,
            tc.tile_pool(name="dram", bufs=1, space="DRAM") as dram,
            tc.tile_pool(name="psA", bufs=6, space="PSUM") as psA,
            tc.tile_pool(name="psM", bufs=2, space="PSUM") as psM,
        ):
            # ---- t=0: warm the PE and pre-pay the collective rendezvous ----
            warm = warmp.tile([128, 512], bf16, tag="warm")
            nc.gpsimd.memset(warm[:].bitcast(f32), 0.0)
            wsrc = warmp.tile([1, 64], f32, tag="wsrc")
            nc.gpsimd.memset(wsrc[:], 0.0)
            Mbd = mres.tile([128, 1024], bf16, tag="Mbd")
            nc.gpsimd.memset(Mbd[:].bitcast(f32), 0.0)

            wbin = dram.tile([1, 64], f32, name="wbin")
            wbo = dram.tile([1, 64], f32, name="wbo")
            nc.gpsimd.dma_start(wbin[:], wsrc[:])
            # dummy rendezvous: pre-pays the one-time collective setup (its
            # hops run as soon as the phase-1 bulk drains); nothing but the
            # real trigger may sit behind it on the gpsimd queue
            nc.gpsimd.collective_compute(
                "AllReduce", mybir.AluOpType.add, replica_groups=GROUPS,
                ins=[wbin.opt()], outs=[wbo.opt()],
            )

            psw = psM.tile([128, 512], f32, tag="mp", name="warm_ps")
            for _ in range(12):
                nc.tensor.matmul(psw[:], warm[:, 0:128], warm[:],
                                 start=True, stop=True)

            # ---- phase-1 input DMAs (6MB; wp deferred to the scalar queue) ----
            xsb = xp.tile([128, 4096], bf16, tag="x")  # col = e*512 + tok
            KV_SLOT = [0, 2, 1, 3]  # stream order k0, v0, k1, v1 -> kvsb col slot
            wkv = [wkvp.tile([128, 4096], bf16, tag=f"wkv{s}", name=f"wkv{s}")
                   for s in range(4)]
            wq = [wqp.tile([128, 4096], bf16, tag=f"wq{h}", name=f"wq{h}")
                  for h in range(2)]
            wp = wpp.tile([128, 8192], bf16, tag="wp")

            nc.sync.dma_start(xsb[:, 0:2048], xd[:, 0:2048])
            nc.sync.dma_start(wkv[0][:], wkvd[0:128, :])
            nc.sync.dma_start(xsb[:, 2048:4096], xd[:, 2048:4096])
            for s in range(1, 4):
                nc.sync.dma_start(wkv[s][:], wkvd[128 * s:128 * (s + 1), :])
            for h in range(2):
                nc.sync.dma_start(wq[h][:], wqd[128 * h:128 * (h + 1), :])

            # kvsb[tt]: [128 tok, 2048] cols = [k(1024) | v(1024)] grouped feats
            kvsb = [kvp.tile([128, 2048], bf16, tag=f"kv{tt}", name=f"kv{tt}")
                    for tt in range(4)]

            def kv_quarter(s):
                slot = KV_SLOT[s]
                for tt in range(4):
                    ps = psA.tile([128, 512], f32, tag="big")
                    for e in range(8):
                        nc.tensor.matmul(
                            ps[:],
                            xsb[:, 512 * e + 128 * tt:512 * e + 128 * (tt + 1)],
                            wkv[s][:, 512 * e:512 * (e + 1)],
                            start=(e == 0), stop=(e == 7),
                        )
                    evict(kvsb[tt][:, 512 * slot:512 * (slot + 1)], ps[:])

            # Msb: both halves' diagonal blocks side by side [128, 2*256] bf16
            Msb = mres.tile([128, 512], bf16, tag="Msb")

            def m_half(g):
                # M blocks 4g..4g+3 (2 heads per 128-block, diagonal 64x64s)
                mp = psM.tile([128, 512], f32, tag="mp", name=f"mp{g}")
                for j in range(4):
                    blk = 4 * g + j
                    for tt in range(4):
                        nc.tensor.matmul(
                            mp[:, 128 * j:128 * (j + 1)],
                            kvsb[tt][:, 128 * blk:128 * (blk + 1)],
                            kvsb[tt][:, 1024 + 128 * blk:1024 + 128 * (blk + 1)],
                            start=(tt == 0), stop=(tt == 3),
                        )
                # extract the 8 diagonal 64x64 blocks -> Msb[:, 256g:256g+256]
                mpv = mp[:].rearrange("p (j c) -> p j c", j=4)
                msv = Msb[:, 256 * g:256 * (g + 1)].rearrange("p (j c) -> p j c", j=4)
                nc.vector.tensor_copy(msv[0:64], mpv[0:64, :, 0:64])
                nc.scalar.copy(msv[64:128], mpv[64:128, :, 64:128])
                if g == 0:
                    # deferred wp half rides the scalar queue (issue-only);
                    # the g=1 half issues after the bounce DMA below
                    nc.scalar.dma_start(wp[:, 0:4096], wpd[:, 0:4096])

            # ---- kv + M halves; one AllReduce for the full M ----
            kv_quarter(0)      # k0
            kv_quarter(1)      # v0
            m_half(0)
            kv_quarter(2)      # k1
            kv_quarter(3)      # v1
            m_half(1)

            # bounce rides the scalar queue: the gpsimd queue is blocked
            # behind the dummy trigger until its mesh begins (~drain+13us),
            # which would delay the bounce past M-readiness
            bin_ = dram.tile([128, 512], bf16, name="bin")
            bo = dram.tile([128, 512], bf16, name="bo")
            nc.scalar.dma_start(bin_[:], Msb[:])
            nc.gpsimd.collective_compute(
                "AllReduce", mybir.AluOpType.add, replica_groups=GROUPS,
                ins=[bin_.opt()], outs=[bo.opt()],
            )
            nc.scalar.dma_start(wp[:, 4096:8192], wpd[:, 4096:8192])

            # ---- q (feature-major, [128 qf, 512 tok] per block), overlaps CC ----
            qsb = [qp.tile([128, TPC], bf16, tag=f"q{f}", name=f"q{f}")
                   for f in range(8)]
            for fq in range(8):
                wqh = wq[fq // 4]
                l = fq % 4
                ps = psA.tile([128, 512], f32, tag="big")
                for e in range(8):
                    nc.tensor.matmul(
                        ps[:],
                        wqh[:, 512 * e + 128 * l:512 * e + 128 * (l + 1)],
                        xsb[:, 512 * e:512 * (e + 1)],
                        start=(e == 0), stop=(e == 7),
                    )
                evict(qsb[fq][:], ps[:])

            # PE filler while the AllReduce completes (keeps the p-state up)
            dum = psA.tile([128, 512], f32, tag="big", name="dum")
            for _ in range(16):
                nc.tensor.matmul(dum[:], warm[:, 0:128], warm[:],
                                 start=True, stop=True)

            # summed M lands straight in Mbd's diagonal spots (zeroed bg);
            # the two return DMAs ride different queues to issue in parallel
            mbv = Mbd[:].rearrange("p (g j c) -> p g j c", g=2, j=4)
            bov = bo[:].rearrange("p (g j c) -> p g j c", g=2, j=4)
            nc.sync.dma_start(mbv[0:64, :, :, 0:64], bov[0:64])
            nc.scalar.dma_start(mbv[64:128, :, :, 64:128], bov[64:128])

            # ---- att blk = Mbd_blk.T @ q_blk (in-place into the q tiles) ----
            for blk in range(8):
                ps = psA.tile([128, 512], f32, tag="big", name=f"att{blk}")
                nc.tensor.matmul(ps[:], Mbd[:, 128 * blk:128 * (blk + 1)],
                                 qsb[blk][:], start=True, stop=True)
                evict(qsb[blk][:], ps[:])
            attsb = qsb

            # ---- out = attT.T @ wp ([512 tok, 1024 o]), stores on 2 queues ----
            st_i = 0
            for oc in range(2):
                for tt in range(4):
                    ps = psA.tile([128, 512], f32, tag="big", name=f"o{oc}_{tt}")
                    for f in range(8):
                        nc.tensor.matmul(
                            ps[:],
                            attsb[f][:, 128 * tt:128 * (tt + 1)],
                            wp[:, 4096 * oc + 512 * f:4096 * oc + 512 * (f + 1)],
                            start=(f == 0), stop=(f == 7),
                        )
                    ot = op.tile([128, 512], f32, tag="osb")
                    evict(ot[:], ps[:])
                    eng = nc.sync if st_i % 2 else nc.gpsimd
                    st_i += 1
                    eng.dma_start(
                        out[128 * tt:128 * (tt + 1), 512 * oc:512 * (oc + 1)],
                        ot[:],
                    )

    nc.compile()
    _built = nc
    return nc


LAST_RESULTS = None  # BassKernelResults of the most recent kernel() call


def _swz(a: np.ndarray) -> np.ndarray:
    """[1024, C] -> [128, 8*C]: row e*128+p -> partition p, cols e*C..e*C+C."""
    C = a.shape[1]
    return np.ascontiguousarray(
        a.reshape(8, 128, C).transpose(1, 0, 2).reshape(128, 8 * C))


def kernel(x: np.ndarray, W_qkv: np.ndarray, W_proj: np.ndarray) -> np.ndarray:
    global LAST_RESULTS
    from ml_dtypes import bfloat16
    from concourse import bass_utils

    nc = _build()

    x = np.ascontiguousarray(x, dtype=np.float32)
    W_qkv = np.ascontiguousarray(W_qkv, dtype=np.float32)
    W_proj = np.ascontiguousarray(W_proj, dtype=np.float32)

    # head-grouping permutation: grouped feature h*64+j <- original row j*16+h
    perm = np.arange(E).reshape(HD, NH).T.ravel()
    Wq_g = W_qkv[perm].astype(bfloat16)
    Wk_g = (W_qkv[E + perm] * np.float32(HD ** -0.5)).astype(bfloat16)  # exact 1/8
    Wv_g = W_qkv[2 * E + perm].astype(bfloat16)
    Wp_g = W_proj.astype(bfloat16)  # att concat order == grouped order already

    # kv stream groups k0, v0, k1, v1: each [512 kvf, 1024 xf] -> swz([1024, 512])
    kv_groups = [Wk_g[0:512], Wv_g[0:512], Wk_g[512:1024], Wv_g[512:1024]]
    wkvd_np = np.concatenate([_swz(np.ascontiguousarray(g.T)) for g in kv_groups], 0)
    wqd_np = np.concatenate(
        [_swz(np.ascontiguousarray(Wq_g[512 * h:512 * (h + 1)].T)) for h in range(2)], 0)
    # wp oc-major: [128 p(af in f), oc*4096 + f*512 + c], wp[p, ...] = Wp[o, af]
    wpT = np.ascontiguousarray(Wp_g.T)           # [1024 af, 1024 o]
    w = wpT.reshape(8, 128, 2, 512)              # [f, p, oc, c]
    wpd_np = np.ascontiguousarray(
        w.transpose(1, 2, 0, 3).reshape(128, 8192))  # [p, oc, f, c]

    in_maps = []
    for c in range(N_CORES):
        b, half = c // 2, c % 2
        xd_c = _swz(np.ascontiguousarray(
            x[b, half * TPC:(half + 1) * TPC, :].T.astype(bfloat16)))
        in_maps.append({"xd": xd_c, "wkvd": wkvd_np, "wqd": wqd_np, "wpd": wpd_np})

    import os as _os
    _tc = _os.environ.get("KERNEL_TRACE_CORES")
    _kw = {"trace_cores": [int(v) for v in _tc.split(",")]} if _tc else {}
    res = bass_utils.run_bass_kernel_spmd(nc, in_maps, core_ids=list(range(N_CORES)), **_kw)
    LAST_RESULTS = res

    out = np.empty((B, T, E), dtype=np.float32)
    for c in range(N_CORES):
        b, half = c // 2, c % 2
        out[b, half * TPC:(half + 1) * TPC, :] = res.results[c]["out"]
    return out


# revision 27
# speedup vs baseline: 1.0986x; 1.0986x over previous
"""Multi-head attention (no softmax) on 8 trn2 NeuronCores.

Reference: out = ((x @ Wqkv.T -> q,k,v per head) ; (q @ k.T * s) @ v ; concat ; @ Wproj.T)

Because there is no softmax the attention is linear:
    (q @ k.T) @ v == q @ (k.T @ v),  k.T @ v is only 64x64 per head,
so the T x T score matrices never need to exist. Per head:
    M_h = (s * k_h).T @ v_h        (64 x 64, reduced over ALL tokens of the batch)
    out += (q_h @ M_h) @ Wproj_h.T

Sharding: token-parallel. Core c owns batch b=c//2, token half c%2 (512 tokens).
M_h needs a reduction over the full batch -> one 128KB AllReduce(add) between
the two cores of each batch.

Everything runs in bf16 (same PE rate as fp32r, half the HBM traffic; rel err
~5e-3 vs the 2e-2 gate). PSUM accumulates fp32. The 1/8 head scale is folded
into W_k on the host (exact).

Collective physics on this platform (measured): every collective op is a
global 8-core rendezvous; the FIRST op carries ~13us of semaphore hops that
freeze while the DMA engines are saturated, and its duration absorbs the
skew between cores; subsequent ops start ~1-2us after the previous and take
~6-10us. So: a dummy 256B AllReduce is triggered at t~8 to pre-pay the
rendezvous (its hops run right after the 6MB phase-1 bulk drains at ~32us),
and the real M AllReduce (both halves at once) chains behind it warm.
W_proj (2MB) is deferred to the scalar queue so the phase-1 drain is early.

DMAs are coarse - the host pre-swizzles every operand into its exact
[128, cols] SBUF layout so each logical group is ONE contiguous dma_start
(the Sync sequencer spends ~0.6us of issue time per dma_start; the f32
baseline burned ~40us there on 64 transfers). Dummy matmuls at t=0 and
during the collective wait keep the PE out of its low p-state.
"""

import numpy as np

B, T, E = 4, 1024, 1024
NH, HD = 16, 64
N_CORES = 8
TPC = T // 2  # tokens per core = 512

_built = None


def _build():
    """Build + compile the 8-core SPMD Bass program once."""
    global _built
    if _built is not None:
        return _built

    import concourse.mybir as mybir
    import concourse.tile as tile
    from concourse import bacc

    f32 = mybir.dt.float32
    bf16 = mybir.dt.bfloat16
    GROUPS = [[0, 1], [2, 3], [4, 5], [6, 7]]

    nc = bacc.Bacc("TRN2", target_bir_lowering=False, debug=False, num_devices=N_CORES)
    # x pre-swizzled: [128 part, e*512 + tok]
    xd = nc.dram_tensor("xd", [128, 4096], bf16, kind="ExternalInput").ap()
    # kv weights: 4 stream groups (k0, v0, k1, v1), each [128 part, e*512 + kvf]
    wkvd = nc.dram_tensor("wkvd", [4 * 128, 8 * 512], bf16, kind="ExternalInput").ap()
    # q weights: 2 column-half groups, each [128 part, e*512 + qf]
    wqd = nc.dram_tensor("wqd", [2 * 128, 8 * 512], bf16, kind="ExternalInput").ap()
    # proj weights, oc-major: [128 part, oc*4096 + f*512 + c]
    wpd = nc.dram_tensor("wpd", [128, 8 * 1024], bf16, kind="ExternalInput").ap()
    out = nc.dram_tensor("out", [TPC, E], f32, kind="ExternalOutput").ap()

    evict_i = [0]

    def evict(dst, src):
        # spread PSUM->SBUF eviction copies across DVE and ACT
        if evict_i[0] % 2 == 0:
            nc.vector.tensor_copy(dst, src)
        else:
            nc.scalar.copy(dst, src)
        evict_i[0] += 1

    with tile.TileContext(nc) as tc:
        with (
            tc.tile_pool(name="xp", bufs=1) as xp,
            tc.tile_pool(name="wkvp", bufs=1) as wkvp,
            tc.tile_pool(name="kvp", bufs=1) as kvp,
            tc.tile_pool(name="wqp", bufs=1) as wqp,
            tc.tile_pool(name="wpp", bufs=1) as wpp,
            tc.tile_pool(name="qp", bufs=1) as qp,
            tc.tile_pool(name="mres", bufs=1) as mres,
            tc.tile_pool(name="op", bufs=2) as op,
            tc.tile_pool(name="warm", bufs=1) as warmp,
            tc.tile_pool(name="dram", bufs=1, space="DRAM") as dram,
            tc.tile_pool(name="psA", bufs=6, space="PSUM") as psA,
            tc.tile_pool(name="psM", bufs=2, space="PSUM") as psM,
        ):
            # ---- t=0: warm the PE and pre-pay the collective rendezvous ----
            warm = warmp.tile([128, 512], bf16, tag="warm")
            nc.gpsimd.memset(warm[:].bitcast(f32), 0.0)
            wsrc = warmp.tile([1, 64], f32, tag="wsrc")
            nc.gpsimd.memset(wsrc[:], 0.0)
            Mbd = mres.tile([128, 1024], bf16, tag="Mbd")
            nc.gpsimd.memset(Mbd[:].bitcast(f32), 0.0)

            wbin = dram.tile([1, 64], f32, name="wbin")
            wbo = dram.tile([1, 64], f32, name="wbo")
            nc.gpsimd.dma_start(wbin[:], wsrc[:])
            # dummy rendezvous: pre-pays the one-time collective setup (its
            # hops run as soon as the phase-1 bulk drains); nothing but the
            # real trigger may sit behind it on the gpsimd queue
            nc.gpsimd.collective_compute(
                "AllReduce", mybir.AluOpType.add, replica_groups=GROUPS,
                ins=[wbin.opt()], outs=[wbo.opt()],
            )

            psw = psM.tile([128, 512], f32, tag="mp", name="warm_ps")
            for _ in range(12):
                nc.tensor.matmul(psw[:], warm[:, 0:128], warm[:],
                                 start=True, stop=True)

            # ---- phase-1 input DMAs (6MB; wp deferred to the scalar queue) ----
            xsb = xp.tile([128, 4096], bf16, tag="x")  # col = e*512 + tok
            KV_SLOT = [0, 2, 1, 3]  # stream order k0, v0, k1, v1 -> kvsb col slot
            wkv = [wkvp.tile([128, 4096], bf16, tag=f"wkv{s}", name=f"wkv{s}")
                   for s in range(4)]
            wq = [wqp.tile([128, 4096], bf16, tag=f"wq{h}", name=f"wq{h}")
                  for h in range(2)]
            wp = wpp.tile([128, 8192], bf16, tag="wp")

            nc.sync.dma_start(wkv[0][:], wkvd[0:128, :])
            nc.sync.dma_start(xsb[:, 0:2048], xd[:, 0:2048])
            nc.sync.dma_start(xsb[:, 2048:4096], xd[:, 2048:4096])
            for s in range(1, 4):
                nc.sync.dma_start(wkv[s][:], wkvd[128 * s:128 * (s + 1), :])
            for h in range(2):
                nc.sync.dma_start(wq[h][:], wqd[128 * h:128 * (h + 1), :])

            # kvsb[tt]: [128 tok, 2048] cols = [k(1024) | v(1024)] grouped feats
            kvsb = [kvp.tile([128, 2048], bf16, tag=f"kv{tt}", name=f"kv{tt}")
                    for tt in range(4)]

            def kv_quarter(s):
                slot = KV_SLOT[s]
                for tt in range(4):
                    ps = psA.tile([128, 512], f32, tag="big")
                    for e in range(8):
                        nc.tensor.matmul(
                            ps[:],
                            xsb[:, 512 * e + 128 * tt:512 * e + 128 * (tt + 1)],
                            wkv[s][:, 512 * e:512 * (e + 1)],
                            start=(e == 0), stop=(e == 7),
                        )
                    evict(kvsb[tt][:, 512 * slot:512 * (slot + 1)], ps[:])

            # Msb: both halves' diagonal blocks side by side [128, 2*256] bf16
            Msb = mres.tile([128, 512], bf16, tag="Msb")

            def m_half(g):
                # M blocks 4g..4g+3 (2 heads per 128-block, diagonal 64x64s)
                mp = psM.tile([128, 512], f32, tag="mp", name=f"mp{g}")
                for j in range(4):
                    blk = 4 * g + j
                    for tt in range(4):
                        nc.tensor.matmul(
                            mp[:, 128 * j:128 * (j + 1)],
                            kvsb[tt][:, 128 * blk:128 * (blk + 1)],
                            kvsb[tt][:, 1024 + 128 * blk:1024 + 128 * (blk + 1)],
                            start=(tt == 0), stop=(tt == 3),
                        )
                # extract the 8 diagonal 64x64 blocks -> Msb[:, 256g:256g+256]
                mpv = mp[:].rearrange("p (j c) -> p j c", j=4)
                msv = Msb[:, 256 * g:256 * (g + 1)].rearrange("p (j c) -> p j c", j=4)
                nc.vector.tensor_copy(msv[0:64], mpv[0:64, :, 0:64])
                nc.scalar.copy(msv[64:128], mpv[64:128, :, 64:128])
                # deferred wp half rides the scalar queue here (issue-only)
                nc.scalar.dma_start(wp[:, 4096 * g:4096 * (g + 1)],
                                    wpd[:, 4096 * g:4096 * (g + 1)])

            # ---- kv + M halves; one AllReduce for the full M ----
            kv_quarter(0)      # k0
            kv_quarter(1)      # v0
            m_half(0)
            kv_quarter(2)      # k1
            kv_quarter(3)      # v1
            m_half(1)

            # bounce rides the scalar queue: the gpsimd queue is blocked
            # behind the dummy trigger until its mesh begins (~drain+13us),
            # which would delay the bounce past M-readiness
            bin_ = dram.tile([128, 512], bf16, name="bin")
            bo = dram.tile([128, 512], bf16, name="bo")
            nc.scalar.dma_start(bin_[:], Msb[:])
            nc.gpsimd.collective_compute(
                "AllReduce", mybir.AluOpType.add, replica_groups=GROUPS,
                ins=[bin_.opt()], outs=[bo.opt()],
            )

            # ---- q (feature-major, [128 qf, 512 tok] per block), overlaps CC ----
            qsb = [qp.tile([128, TPC], bf16, tag=f"q{f}", name=f"q{f}")
                   for f in range(8)]
            for fq in range(8):
                wqh = wq[fq // 4]
                l = fq % 4
                ps = psA.tile([128, 512], f32, tag="big")
                for e in range(8):
                    nc.tensor.matmul(
                        ps[:],
                        wqh[:, 512 * e + 128 * l:512 * e + 128 * (l + 1)],
                        xsb[:, 512 * e:512 * (e + 1)],
                        start=(e == 0), stop=(e == 7),
                    )
                evict(qsb[fq][:], ps[:])

            # PE filler while the AllReduce completes (keeps the p-state up)
            dum = psA.tile([128, 512], f32, tag="big", name="dum")
            for _ in range(16):
                nc.tensor.matmul(dum[:], warm[:, 0:128], warm[:],
                                 start=True, stop=True)

            # summed M lands straight in Mbd's diagonal spots (zeroed bg);
            # the two return DMAs ride different queues to issue in parallel
            mbv = Mbd[:].rearrange("p (g j c) -> p g j c", g=2, j=4)
            bov = bo[:].rearrange("p (g j c) -> p g j c", g=2, j=4)
            nc.sync.dma_start(mbv[0:64, :, :, 0:64], bov[0:64])
            nc.scalar.dma_start(mbv[64:128, :, :, 64:128], bov[64:128])

            # ---- att blk = Mbd_blk.T @ q_blk (in-place into the q tiles) ----
            for blk in range(8):
                ps = psA.tile([128, 512], f32, tag="big", name=f"att{blk}")
                nc.tensor.matmul(ps[:], Mbd[:, 128 * blk:128 * (blk + 1)],
                                 qsb[blk][:], start=True, stop=True)
                evict(qsb[blk][:], ps[:])
            attsb = qsb

            # ---- out = attT.T @ wp ([512 tok, 1024 o]), stores on 2 queues ----
            st_i = 0
            for oc in range(2):
                for tt in range(4):
                    ps = psA.tile([128, 512], f32, tag="big", name=f"o{oc}_{tt}")
                    for f in range(8):
                        nc.tensor.matmul(
                            ps[:],
                            attsb[f][:, 128 * tt:128 * (tt + 1)],
                            wp[:, 4096 * oc + 512 * f:4096 * oc + 512 * (f + 1)],
                            start=(f == 0), stop=(f == 7),
                        )
                    ot = op.tile([128, 512], f32, tag="osb")
                    evict(ot[:], ps[:])
                    eng = nc.sync if st_i % 2 else nc.gpsimd
                    st_i += 1
                    eng.dma_start(
                        out[128 * tt:128 * (tt + 1), 512 * oc:512 * (oc + 1)],
                        ot[:],
                    )

    nc.compile()
    _built = nc
    return nc


LAST_RESULTS = None  # BassKernelResults of the most recent kernel() call


def _swz(a: np.ndarray) -> np.ndarray:
    """[1024, C] -> [128, 8*C]: row e*128+p -> partition p, cols e*C..e*C+C."""
    C = a.shape[1]
    return np.ascontiguousarray(
        a.reshape(8, 128, C).transpose(1, 0, 2).reshape(128, 8 * C))


def kernel(x: np.ndarray, W_qkv: np.ndarray, W_proj: np.ndarray) -> np.ndarray:
    global LAST_RESULTS
    from ml_dtypes import bfloat16
    from concourse import bass_utils

    nc = _build()

    x = np.ascontiguousarray(x, dtype=np.float32)
    W_qkv = np.ascontiguousarray(W_qkv, dtype=np.float32)
    W_proj = np.ascontiguousarray(W_proj, dtype=np.float32)

    # head-grouping permutation: grouped feature h*64+j <- original row j*16+h
    perm = np.arange(E).reshape(HD, NH).T.ravel()
    Wq_g = W_qkv[perm].astype(bfloat16)
    Wk_g = (W_qkv[E + perm] * np.float32(HD ** -0.5)).astype(bfloat16)  # exact 1/8
    Wv_g = W_qkv[2 * E + perm].astype(bfloat16)
    Wp_g = W_proj.astype(bfloat16)  # att concat order == grouped order already

    # kv stream groups k0, v0, k1, v1: each [512 kvf, 1024 xf] -> swz([1024, 512])
    kv_groups = [Wk_g[0:512], Wv_g[0:512], Wk_g[512:1024], Wv_g[512:1024]]
    wkvd_np = np.concatenate([_swz(np.ascontiguousarray(g.T)) for g in kv_groups], 0)
    wqd_np = np.concatenate(
        [_swz(np.ascontiguousarray(Wq_g[512 * h:512 * (h + 1)].T)) for h in range(2)], 0)
    # wp oc-major: [128 p(af in f), oc*4096 + f*512 + c], wp[p, ...] = Wp[o, af]
    wpT = np.ascontiguousarray(Wp_g.T)           # [1024 af, 1024 o]
    w = wpT.reshape(8, 128, 2, 512)              # [f, p, oc, c]
    wpd_np = np.ascontiguousarray(
        w.transpose(1, 2, 0, 3).reshape(128, 8192))  # [p, oc, f, c]

    in_maps = []
    for c in range(N_CORES):
        b, half = c // 2, c % 2
        xd_c = _swz(np.ascontiguousarray(
            x[b, half * TPC:(half + 1) * TPC, :].T.astype(bfloat16)))
        in_maps.append({"xd": xd_c, "wkvd": wkvd_np, "wqd": wqd_np, "wpd": wpd_np})

    import os as _os
    _tc = _os.environ.get("KERNEL_TRACE_CORES")
    _kw = {"trace_cores": [int(v) for v in _tc.split(",")]} if _tc else {}
    res = bass_utils.run_bass_kernel_spmd(nc, in_maps, core_ids=list(range(N_CORES)), **_kw)
    LAST_RESULTS = res

    out = np.empty((B, T, E), dtype=np.float32)
    for c in range(N_CORES):
        b, half = c // 2, c % 2
        out[b, half * TPC:(half + 1) * TPC, :] = res.results[c]["out"]
    return out
